# revision 13
# baseline (speedup 1.0000x reference)
"""Trainium2 Bass kernel for the Barrier-Net GNN message-passing problem.

Full (unsharded) inputs in, full output out.  Internally: pure data-parallel
shard of the row axis across 8 NeuronCores; MLP weights replicated.

Math restructure vs the reference (all exact, precomputed on host):
  - phi L3 (linear) + deepset-sum + rho L1 (linear part) collapse into
    accumulating K=128 matmuls with lhsT = [W3@W4; W3@W4].
  - rho L2 (linear) collapses into psi L1: Wna = W5n@W6[:16], etc.
  - The global scalar `val` (row-0 barrier/empty norm ratio) is computed on
    the host (single row) and shipped as a tensor.

Layouts:
  - geometry (P, H, barrier, minH, final scaling) in natural row-major tiles
    with strided access patterns.
  - MLPs in feature-major tiles obtained via PE transpose; matmuls run as
    float32r (1 cycle/row for N>=256) via AP bitcast.
"""

import os
import sys
from contextlib import ExitStack

import numpy as np

sys.path.insert(0, "/opt/trn_rl_repo")

# ---------------------------------------------------------------- constants
ND = 100000
NN = 8
NO = 8
SD = 4
AD = 2
WIDTH = 53
R_AGENT, B_GAMMA, DELTA_R = 0.2, 0.05, 0.5
PHI_MAX, A_MAX = 0.5, 1.0

NCORES = 8
ROWS_CORE = 12800          # padded rows per core  (8*12800 = 102400 >= 100000)
NGROUPS = 5                # groups per core
BLKS_G = 20                # 128-row blocks per group  (128*20 = 2560 rows)
TILES_G = 5                # MLP tiles per group (each tile = 4 blocks = 512 rows)
B = 512                    # rows per MLP tile

F32 = None                 # filled after mybir import
_NC_CACHE = {}
TRACE = False
TRACE_DIR = None
LAST_RESULT = None


# ---------------------------------------------------------------- host math
def _np_mlp(h, params):
    for W, b in params[:-1]:
        h = np.maximum(h @ W + b, 0.0)
    W, b = params[-1]
    return h @ W + b


def _host_val(x0, phi_n, rho_n, phi_o, rho_o, psi):
    """min(||barrier_0|| / ||empty_0||, 1) for row 0, float32 on host."""
    x0 = x0.astype(np.float32)
    nb = 1 + SD * (np.arange(NN) + 1)
    Pn = x0[nb[:, None] + np.arange(2)]                    # [nn,2]
    Hn = np.linalg.norm(Pn, axis=1) - 2.0 * R_AGENT
    ob = 1 + SD * (NN + 1) + 2 * np.arange(NO)
    Po = x0[ob[:, None] + np.arange(2)]                    # [no,2]
    lo, hi = -Po - 0.5, -Po + 0.5
    cp = np.clip(np.zeros_like(Po), lo, hi)
    Ho = np.linalg.norm(cp, axis=1) - R_AGENT
    P = np.concatenate([Pn, Po], 0)
    H = np.concatenate([Hn, Ho], 0)
    normP = np.linalg.norm(P, axis=1, keepdims=True)
    barrier = (-B_GAMMA) * np.sum(P / (H[:, None] * normP), axis=0)
    g = x0[1:1 + SD]
    e_n = _np_mlp(x0[1 + SD:1 + SD + SD * NN].reshape(NN, SD), phi_n).sum(0)
    rn = _np_mlp(e_n, rho_n)
    e_o = _np_mlp(x0[1 + SD + SD * NN:].reshape(NO, 2), phi_o).sum(0)
    ro = _np_mlp(e_o, rho_o)
    pin = np.concatenate([rn, ro, g], 0)
    empty = _np_mlp(pin, psi)
    inv = max(np.linalg.norm(empty) / PHI_MAX, 1.0)
    empty = empty / inv
    normb = np.linalg.norm(barrier)
    normpi = np.linalg.norm(empty)
    return np.float32(min(normb / normpi, 1.0))


def _prep_consts(phi_n, rho_n, phi_o, rho_o, psi):
    """Build the fused / stacked weight tensors (host, float32)."""
    f = np.float32
    W1n, b1n = phi_n[0]
    W2n, b2n = phi_n[1]
    W3n, b3n = phi_n[2]
    W1o, b1o = phi_o[0]
    W2o, b2o = phi_o[1]
    W3o, b3o = phi_o[2]
    W4n, b4n = rho_n[0]
    W5n, b5n = rho_n[1]
    W4o, b4o = rho_o[0]
    W5o, b5o = rho_o[1]
    W6, b6 = psi[0]
    W7, b7 = psi[1]
    W8, b8 = psi[2]

    c = {}
    # L1 fused: lhsT [53, 128] per element pair (cols 0-63 el 2q, 64-127 el 2q+1)
    wl1n = np.zeros((53, 512), f)
    for j in range(NN):
        r = 5 + 4 * j
        wl1n[r:r + 4, 64 * j:64 * j + 64] = W1n
    c["wl1n"] = wl1n
    wl1o = np.zeros((53, 512), f)
    for j in range(NO):
        r = 37 + 2 * j
        wl1o[r:r + 2, 64 * j:64 * j + 64] = W1o
    c["wl1o"] = wl1o
    # L2 block-diagonal [128, 128]
    wl2n = np.zeros((128, 128), f)
    wl2n[:64, :64] = W2n
    wl2n[64:, 64:] = W2n
    c["wl2n"] = wl2n
    wl2o = np.zeros((128, 128), f)
    wl2o[:64, :64] = W2o
    wl2o[64:, 64:] = W2o
    c["wl2o"] = wl2o
    # phi L3 + sum + rho L1 collapse: [W3@W4; W3@W4]  [128, 64]
    w34n = (W3n @ W4n).astype(f)
    c["wl34n"] = np.vstack([w34n, w34n]).astype(f)
    w34o = (W3o @ W4o).astype(f)
    c["wl34o"] = np.vstack([w34o, w34o]).astype(f)
    # rho L2 collapsed into psi L1
    wna = (W5n @ W6[0:16]).astype(f)      # [64, 64]
    wno = (W5o @ W6[16:32]).astype(f)
    c["wpsi1n"] = np.ascontiguousarray(wna)               # [64, 64]
    c["wpsi1o"] = np.ascontiguousarray(wno)
    wpsig = np.zeros((53, 64), f)
    wpsig[1:5, :] = W6[32:36]
    c["wpsig"] = wpsig
    c["wpsi2"] = np.asarray(W7, f)
    c["wpsi3"] = np.asarray(W8, f)                        # [64, 2]
    # biases (column vectors)
    c["bl1n"] = np.concatenate([b1n, b1n])[:, None].astype(f)   # [128,1]
    c["bl1o"] = np.concatenate([b1o, b1o])[:, None].astype(f)
    c["bl2n"] = np.concatenate([b2n, b2n])[:, None].astype(f)
    c["bl2o"] = np.concatenate([b2o, b2o])[:, None].astype(f)
    brn = (b4n + NN * (b3n @ W4n)).astype(f)
    bro = (b4o + NO * (b3o @ W4o)).astype(f)
    c["brho"] = np.concatenate([brn, bro])[:, None].astype(f)   # [128,1]
    c["bpsi1"] = (b6 + b5n @ W6[0:16] + b5o @ W6[16:32])[:, None].astype(f)
    c["bpsi2"] = np.asarray(b7, f)[:, None]
    c["bpsi3"] = np.asarray(b8, f)[:, None]               # [2,1]
    c["ident"] = np.eye(128, dtype=f)
    return c


# ---------------------------------------------------------------- kernel IR
def build_nc(ngroups=NGROUPS, blks_g=BLKS_G, tiles_g=TILES_G, reps=1):
    import concourse.bass as bass
    import concourse.bacc as bacc
    import concourse.tile as tile
    from concourse import mybir

    f32 = mybir.dt.float32
    f32r = mybir.dt.float32r
    ALU = mybir.AluOpType
    ACT = mybir.ActivationFunctionType
    AX = mybir.AxisListType

    rows = ngroups * blks_g * 128
    nc = bacc.Bacc()

    x = nc.declare_dram_parameter("x", [rows, WIDTH], f32, isOutput=False)
    prm = {}
    for name, shape in [
        ("wl1n", [53, 512]), ("wl1o", [53, 512]),
        ("wl2n", [128, 128]), ("wl2o", [128, 128]),
        ("wl34n", [128, 64]), ("wl34o", [128, 64]),
        ("wpsi1n", [64, 64]), ("wpsi1o", [64, 64]), ("wpsig", [53, 64]),
        ("wpsi2", [64, 64]), ("wpsi3", [64, 2]),
        ("bl1n", [128, 1]), ("bl1o", [128, 1]),
        ("bl2n", [128, 1]), ("bl2o", [128, 1]),
        ("brho", [128, 1]), ("bpsi1", [64, 1]), ("bpsi2", [64, 1]),
        ("bpsi3", [2, 1]), ("ident", [128, 128]), ("valm1", [1, 1]),
    ]:
        prm[name] = nc.declare_dram_parameter(name, shape, f32, isOutput=False)
    y = nc.declare_dram_parameter("y", [rows, 2], f32, isOutput=True)

    # row mapping: row = g*(128*blks_g) + p*blks_g + b
    xv = x[:].rearrange("(g p b) c -> g p b c", g=ngroups, p=128, b=blks_g)
    yv = y[:].rearrange("(g p b) c -> g p b c", g=ngroups, p=128, b=blks_g)

    with tile.TileContext(nc) as tc:
        ctx = ExitStack()
        singles = ctx.enter_context(tc.tile_pool(name="singles", bufs=1))
        xpool = ctx.enter_context(tc.tile_pool(name="xpool", bufs=2))
        mpool = ctx.enter_context(tc.tile_pool(name="mpool", bufs=2))
        vpool = ctx.enter_context(tc.tile_pool(name="vpool", bufs=2))
        opool = ctx.enter_context(tc.tile_pool(name="opool", bufs=2))
        ppool = ctx.enter_context(tc.tile_pool(name="ppool", bufs=1, space="PSUM"))

        # ---- load constants
        W = {}
        mm_weights = {"wl1n", "wl1o", "wl2n", "wl2o", "wl34n", "wl34o",
                      "wpsi1n", "wpsi1o", "wpsig", "wpsi2", "wpsi3"}
        for name in ["wl1n", "wl1o", "wl2n", "wl2o", "wl34n", "wl34o",
                     "wpsi1n", "wpsi1o", "wpsig", "wpsi2", "wpsi3",
                     "bl1n", "bl1o", "bl2n", "bl2o", "brho",
                     "bpsi1", "bpsi2", "bpsi3", "ident"]:
            dt_ = f32r if name in mm_weights else f32
            t = singles.tile(list(prm[name].shape), dt_, tag=name)
            src_ap = prm[name][:]
            if name in mm_weights:
                src_ap = src_ap.bitcast(f32r)
            nc.sync.dma_start(out=t, in_=src_ap)
            W[name] = t
        valm1 = singles.tile([128, 1], f32, tag="valm1")
        nc.sync.dma_start(out=valm1, in_=prm["valm1"][:].partition_broadcast(128))
        ones_g = singles.tile([128, blks_g], f32, tag="ones")
        nc.vector.memset(ones_g, 1.0)
        cm05 = singles.tile([128, 1], f32, tag="cm05")
        nc.vector.memset(cm05, -0.5)

        def relu_act(out, in_, bias):
            nc.scalar.activation(out=out, in_=in_, func=ACT.Relu, bias=bias,
                                 scale=1.0)

        def relu_dve(out, in_, bias):
            nc.vector.tensor_scalar(out=out, in0=in_, scalar1=bias,
                                    scalar2=0.0, op0=ALU.add, op1=ALU.max)

        def bcast2(ap):
            # append broadcast (step 0) coord dim of size 2
            return ap[:, :, None].to_broadcast(list(ap.shape) + [2])

        for g in [g for _ in range(reps) for g in range(ngroups)]:
            xg = xpool.tile([128, blks_g, WIDTH], f32, tag="xg")
            nc.sync.dma_start(out=xg, in_=xv[g])
            empg = opool.tile([128, blks_g, 2], f32, tag="empg")

            # ================= MLP path, per 512-row tile =================
            for t in range(tiles_g):
                blk0 = 4 * t
                nblk = min(4, blks_g - blk0)
                bt = 128 * nblk
                # ---- transpose input block to feature-major [53, bt]
                xt_ps = ppool.tile([53, 512], f32, tag="xt")
                for bb in range(nblk):
                    nc.tensor.transpose(
                        out=xt_ps[:, 128 * bb:128 * (bb + 1)],
                        in_=xg[:, blk0 + bb, :],
                        identity=W["ident"][:],
                    )
                xt = mpool.tile([53, 512], f32r, tag="xt_sb")
                nc.scalar.copy(out=xt[:, :bt], in_=xt_ps[:, :bt])
                xtr = xt[:, :bt]

                rho_n_ps = ppool.tile([64, 512], f32, tag="rho_n")
                rho_o_ps = ppool.tile([64, 512], f32, tag="rho_o")
                # ---- the two deepsets
                for s, (wl1, wl2, wl34, bl1, bl2) in enumerate([
                    ("wl1n", "wl2n", "wl34n", "bl1n", "bl2n"),
                    ("wl1o", "wl2o", "wl34o", "bl1o", "bl2o"),
                ]):
                    for q in range(4):
                        pb = ppool.tile([128, 512], f32, tag=f"l1{q % 2}")
                        nc.tensor.matmul(
                            pb[:, :bt],
                            lhsT=W[wl1][:, 128 * q:128 * (q + 1)],
                            rhs=xtr, start=True, stop=True)
                        h1 = mpool.tile([128, 512], f32r, tag=f"h1{q % 2}")
                        if q % 2 == 0:
                            relu_act(h1[:, :bt], pb[:, :bt], W[bl1][:])
                        else:
                            relu_dve(h1[:, :bt], pb[:, :bt], W[bl1][:])
                        p2 = ppool.tile([128, 512], f32, tag="l2")
                        nc.tensor.matmul(
                            p2[:, :bt], lhsT=W[wl2][:],
                            rhs=h1[:, :bt], start=True, stop=True)
                        h2 = mpool.tile([128, 512], f32r, tag=f"h2{q % 2}")
                        if q % 2 == 0:
                            relu_dve(h2[:, :bt], p2[:, :bt], W[bl2][:])
                        else:
                            relu_act(h2[:, :bt], p2[:, :bt], W[bl2][:])
                        # phi L3 + sum + rho L1 (collapsed), accumulate
                        rho_ps = rho_n_ps if s == 0 else rho_o_ps
                        nc.tensor.matmul(
                            rho_ps[:, :bt],
                            lhsT=W[wl34][:],
                            rhs=h2[:, :bt],
                            start=(q == 0), stop=(q == 3))
                # ---- rho relu -> psi
                hn = mpool.tile([64, 512], f32r, tag="hn")
                relu_act(hn[:, :bt], rho_n_ps[:, :bt], W["brho"][0:64])
                ho = mpool.tile([64, 512], f32r, tag="ho")
                relu_act(ho[:, :bt], rho_o_ps[:, :bt], W["brho"][64:128])
                p1 = ppool.tile([64, 512], f32, tag="p1")
                nc.tensor.matmul(p1[:, :bt], lhsT=W["wpsi1n"][:],
                                 rhs=hn[:, :bt], start=True, stop=False)
                nc.tensor.matmul(p1[:, :bt], lhsT=W["wpsi1o"][:],
                                 rhs=ho[:, :bt], start=False, stop=False)
                nc.tensor.matmul(p1[:, :bt], lhsT=W["wpsig"][:],
                                 rhs=xtr, start=False, stop=True)
                hp1 = mpool.tile([64, 512], f32r, tag="hp1")
                relu_dve(hp1[:, :bt], p1[:, :bt], W["bpsi1"][:])
                p2p = ppool.tile([64, 512], f32, tag="p2")
                nc.tensor.matmul(p2p[:, :bt], lhsT=W["wpsi2"][:],
                                 rhs=hp1[:, :bt],
                                 start=True, stop=True)
                hp2 = mpool.tile([64, 512], f32r, tag="hp2")
                relu_act(hp2[:, :bt], p2p[:, :bt], W["bpsi2"][:])
                p3 = ppool.tile([2, 512], f32, tag="p1")
                nc.tensor.matmul(p3[:, :bt], lhsT=W["wpsi3"][:],
                                 rhs=hp2[:, :bt],
                                 start=True, stop=True)
                emp_sb = mpool.tile([2, 512], f32, tag="emp_sb")
                nc.scalar.activation(out=emp_sb[:, :bt], in_=p3[:, :bt],
                                     func=ACT.Identity, bias=W["bpsi3"][:],
                                     scale=1.0)
                # ---- transpose back to natural [128, nblk, 2]
                empT = ppool.tile([128, 4, 2], f32, tag="p2")
                for bb in range(nblk):
                    nc.tensor.transpose(
                        out=empT[:, bb, :],
                        in_=emp_sb[:, 128 * bb:128 * (bb + 1)],
                        identity=W["ident"][0:2, 0:2])
                nc.scalar.copy(out=empg[:, blk0:blk0 + nblk, :],
                               in_=empT[:, :nblk, :])

            # ================= geometry path, whole group =================
            G = blks_g
            xo = xg[:, :, 37:53]                           # [128,G,16] obstacles
            xn4 = xg[:, :, 5:37].rearrange("p b (e d) -> p b e d", d=4)
            PnV = xn4[:, :, :, 0:2]                        # [128,G,8,2]
            PoV = xo.rearrange("p b (e d) -> p b e d", d=2)  # [128,G,8,2]

            # clipped closest point on obstacle squares
            t1 = vpool.tile([128, G, 16], f32, tag="t1")
            nc.scalar.activation(out=t1, in_=xo, func=ACT.Identity,
                                 bias=cm05[:], scale=-1.0)
            t2 = vpool.tile([128, G, 16], f32, tag="t2")
            nc.vector.tensor_scalar_max(t2, t1, 0.0)
            cp = vpool.tile([128, G, 16], f32, tag="cp")
            nc.vector.scalar_tensor_tensor(cp, in0=t1, scalar=1.0, in1=t2,
                                           op0=ALU.add, op1=ALU.min)
            cpV = cp.rearrange("p b (e d) -> p b e d", d=2)

            # squared norms
            sq = vpool.tile([128, G, 8, 2], f32, tag="sq")
            np2 = vpool.tile([128, G, 16], f32, tag="np2")
            nc.vector.tensor_mul(sq, PnV, PnV)
            nc.vector.tensor_reduce(out=np2[:, :, 0:8], in_=sq, axis=AX.X,
                                    op=ALU.add)
            nc.vector.tensor_mul(sq, PoV, PoV)
            nc.vector.tensor_reduce(out=np2[:, :, 8:16], in_=sq, axis=AX.X,
                                    op=ALU.add)
            sqc = vpool.tile([128, G, 8, 2], f32, tag="sqc")
            cq2 = vpool.tile([128, G, 8], f32, tag="cq2")
            nc.vector.tensor_mul(sqc, cpV, cpV)
            nc.vector.tensor_reduce(out=cq2, in_=sqc, axis=AX.X, op=ALU.add)

            normP = vpool.tile([128, G, 16], f32, tag="normP")
            nc.scalar.sqrt(normP, np2)
            hos = vpool.tile([128, G, 8], f32, tag="hos")
            nc.scalar.sqrt(hos, cq2)

            den = vpool.tile([128, G, 16], f32, tag="den")
            nc.vector.scalar_tensor_tensor(
                den[:, :, 0:8], in0=normP[:, :, 0:8], scalar=-2.0 * R_AGENT,
                in1=normP[:, :, 0:8], op0=ALU.add, op1=ALU.mult)
            nc.vector.scalar_tensor_tensor(
                den[:, :, 8:16], in0=hos, scalar=-R_AGENT,
                in1=normP[:, :, 8:16], op0=ALU.add, op1=ALU.mult)
            w = vpool.tile([128, G, 16], f32, tag="w")
            nc.vector.reciprocal(w, den)

            # minH < DELTA_R mask, in squared domain
            mn = vpool.tile([128, G], f32, tag="mn")
            nc.vector.tensor_reduce(out=mn, in_=np2[:, :, 0:8], axis=AX.X,
                                    op=ALU.min)
            mo = vpool.tile([128, G], f32, tag="mo")
            nc.vector.tensor_reduce(out=mo, in_=cq2, axis=AX.X, op=ALU.min)
            thr_n = (DELTA_R + 2.0 * R_AGENT) ** 2
            thr_o = (DELTA_R + R_AGENT) ** 2
            s1 = vpool.tile([128, G], f32, tag="s1")
            nc.vector.tensor_scalar(out=s1, in0=mn, scalar1=thr_n,
                                    scalar2=None, op0=ALU.is_lt)
            s2 = vpool.tile([128, G], f32, tag="s2")
            nc.vector.tensor_scalar(out=s2, in0=mo, scalar1=thr_o,
                                    scalar2=None, op0=ALU.is_lt)
            mask = vpool.tile([128, G], f32, tag="mask")
            nc.vector.tensor_tensor(out=mask, in0=s1, in1=s2, op=ALU.max)
            scal = vpool.tile([128, G], f32, tag="scal")
            nc.vector.scalar_tensor_tensor(scal, in0=mask, scalar=valm1[:],
                                           in1=ones_g[:], op0=ALU.mult,
                                           op1=ALU.add)

            # barrier sums
            pw = vpool.tile([128, G, 16], f32, tag="pw")
            bxy = vpool.tile([128, G, 2], f32, tag="bxy")
            for ci in range(2):
                nc.vector.tensor_mul(pw[:, :, 0:8], xn4[:, :, :, ci],
                                     w[:, :, 0:8])
                nc.vector.tensor_mul(pw[:, :, 8:16], PoV[:, :, :, ci],
                                     w[:, :, 8:16])
                nc.vector.tensor_reduce(out=bxy[:, :, ci], in_=pw, axis=AX.X,
                                        op=ALU.add)

            # empty norm clip to PHI_MAX, times scaling
            sq2 = vpool.tile([128, G, 2], f32, tag="sq2")
            ne2 = vpool.tile([128, G], f32, tag="ne2")
            nc.vector.tensor_mul(sq2, empg, empg)
            nc.vector.tensor_reduce(out=ne2, in_=sq2, axis=AX.X, op=ALU.add)
            nrm = vpool.tile([128, G], f32, tag="nrm")
            nc.scalar.sqrt(nrm, ne2)
            rcp = vpool.tile([128, G], f32, tag="rcp")
            nc.vector.reciprocal(rcp, nrm)
            fs = vpool.tile([128, G], f32, tag="fs")
            nc.vector.tensor_scalar(out=fs, in0=rcp, scalar1=PHI_MAX,
                                    scalar2=1.0, op0=ALU.mult, op1=ALU.min)
            nc.vector.tensor_mul(fs, fs, scal)
            em2 = vpool.tile([128, G, 2], f32, tag="em2")
            nc.vector.tensor_mul(em2, empg, bcast2(fs[:]))

            # t = -B_GAMMA*bxy + em2 ; clip norm to A_MAX
            tt = opool.tile([128, G, 2], f32, tag="tt")
            nc.vector.scalar_tensor_tensor(tt, in0=bxy, scalar=-B_GAMMA,
                                           in1=em2, op0=ALU.mult, op1=ALU.add)
            nc.vector.tensor_mul(sq2, tt, tt)
            nc.vector.tensor_reduce(out=ne2, in_=sq2, axis=AX.X, op=ALU.add)
            nc.scalar.sqrt(nrm, ne2)
            nc.vector.reciprocal(rcp, nrm)
            f2 = vpool.tile([128, G], f32, tag="f2")
            nc.vector.tensor_scalar(out=f2, in0=rcp, scalar1=A_MAX,
                                    scalar2=1.0, op0=ALU.mult, op1=ALU.min)
            outv = opool.tile([128, G, 2], f32, tag="outv")
            nc.vector.tensor_mul(outv, tt, bcast2(f2[:]))
            nc.sync.dma_start(out=yv[g], in_=outv)

        ctx.close()
    nc.compile()
    return nc


# ---------------------------------------------------------------- entrypoint
def kernel(x, phi_n, rho_n, phi_o, rho_o, psi, nn_count=None, no_count=None,
           **_unused):
    from concourse.bass_utils import run_bass_kernel_spmd

    def _np(params):
        return [(np.asarray(w, np.float32), np.asarray(b, np.float32))
                for (w, b) in params]

    x = np.asarray(x, np.float32)
    phi_n, rho_n, phi_o, rho_o, psi = map(_np, (phi_n, rho_n, phi_o, rho_o, psi))

    consts = _prep_consts(phi_n, rho_n, phi_o, rho_o, psi)
    val = _host_val(x[0], phi_n, rho_n, phi_o, rho_o, psi)
    consts["valm1"] = np.array([[val - 1.0]], np.float32)

    # pad and shard
    total = NCORES * ROWS_CORE
    xp = np.concatenate([x, np.broadcast_to(x[0], (total - ND, WIDTH))], 0)
    xp = np.ascontiguousarray(xp, np.float32)

    key = "full"
    if key not in _NC_CACHE:
        _NC_CACHE[key] = build_nc()
    nc = _NC_CACHE[key]

    in_maps = []
    for c in range(NCORES):
        m = {"x": xp[c * ROWS_CORE:(c + 1) * ROWS_CORE]}
        m.update(consts)
        in_maps.append(m)

    kw = {}
    if TRACE:
        kw = dict(trace=True, trace_cores=[0], tmpdir=TRACE_DIR)
    res = run_bass_kernel_spmd(nc, in_maps, core_ids=list(range(NCORES)), **kw)
    if TRACE:
        global LAST_RESULT
        LAST_RESULT = res
    out = np.concatenate([res.results[c]["y"] for c in range(NCORES)], 0)
    return np.ascontiguousarray(out[:ND]).astype(np.float32)


if __name__ == "__main__":
    # smoke build
    nc = build_nc(ngroups=1, blks_g=4, tiles_g=1)
    print("built ok")
